# revision 3
# baseline (speedup 1.0000x reference)
"""Masked attention kernel for Trainium2, 8 NeuronCores.

Problem: q,k,v [32,1024,64] f32, mask [32,1024,1024] bool (True -> -inf),
out = softmax(q@k^T * D^-0.5 masked) @ v.

Sharding: batch*head dim (32) split across 8 cores, 4 heads/core.

v2 design (ACT-clocked): the scalar engine's 32 exps (one per score tile,
~1us each) are the critical resource, so ACT does ONLY exp.  Mask handling
is split per tile class:
  - PE-class tiles: additive -240*mask (exact in fp8) is accumulated into
    the score PSUM by a DoubleRow fp8 identity matmul (0.5 cyc/row), so the
    tile reaches exp fully masked.
  - DVE-class tiles: multiplicative bf16 keep {1,0} applied post-exp by DVE
    tensor_mul (2-byte operands hit the 2x_1p DVE mode).
QK/AV run in bf16 (fp8 fails the 2e-2 tolerance; measured).  AV uses the
v_aug ones-row to carry softmax denominators.  Tail: DVE cast, PE transpose
(identity65), DVE reciprocal+broadcast-mul, bf16 out-DMA (host casts f32).

Startup: a gpsimd memset feeds an immediate ACT exp (loads the Exp table at
~6.5us instead of ~11us) and a chain of junk PE matmuls keeps the tensor
engine busy while input DMAs land, so it reaches max p-state (0.42ns/col
instead of 0.83) before the first real QK.  Input DMAs are hoisted into the
preamble to overlap the engine-table loads.
"""

import os
import sys

import numpy as np

for _p in ("/opt/trn_rl_repo", "/opt/pypackages"):
    if _p not in sys.path and os.path.isdir(_p):
        sys.path.append(_p)

import ml_dtypes  # noqa: E402

import concourse.bass as bass  # noqa: E402
import concourse.tile as tile  # noqa: E402
from concourse import mybir  # noqa: E402
from concourse.bass_utils import run_bass_kernel_spmd  # noqa: E402

BH, S, D = 32, 1024, 64
NCORES = 8
HPC = BH // NCORES  # heads per core
NT = S // 128  # 8 tiles of 128 along s/t
FP8 = ml_dtypes.float8_e4m3fn
F32 = mybir.dt.float32
BF16 = mybir.dt.bfloat16
DT8 = mybir.dt.float8e4
DR = mybir.MatmulPerfMode.DoubleRow

# per-head tile classes: tiles < NPE get the additive fp8 PE mask (DoubleRow
# identity accumulate); the rest get the multiplicative bf16 DVE mask.
NPE = 4
WARM_N = 28  # junk PE matmuls to hold p-state while input DMAs land


def _build_program():
    nc = bass.Bass(
        "TRN2",
        target_bir_lowering=False,
        debug=False,
        num_devices=NCORES,
    )
    # qkt[h]: rows 0-63 = kT | qT side by side: [64, NT*128 (kT) + S (qT)]
    qkt = nc.dram_tensor("qkt", [HPC, 64, 2 * S], BF16, kind="ExternalInput").ap()
    vaug = nc.dram_tensor("vaug", [HPC, 128, NT * 80], BF16, kind="ExternalInput").ap()
    # additive masks, DR-interleaved: per PE-class tile [64, 2, 1024] fp8
    mpe = nc.dram_tensor("mpe", [HPC, 64, NPE * 2 * S], DT8, kind="ExternalInput").ap()
    # multiplicative keep masks, bf16 [128, 1024] per DVE-class tile
    mdv = nc.dram_tensor(
        "mdv", [HPC, 128, (NT - NPE) * S], BF16, kind="ExternalInput"
    ).ap()
    # identity for the tail transposes + DR identity for mask accumulate
    ident = nc.dram_tensor("ident", [65, 65], BF16, kind="ExternalInput").ap()
    iddr = nc.dram_tensor("iddr", [64, 256], DT8, kind="ExternalInput").ap()
    outp = nc.dram_tensor("outp", [HPC, 128, NT * D], BF16, kind="ExternalOutput").ap()

    with tile.TileContext(nc) as tc:
        with (
            tc.tile_pool(name="const", bufs=1) as const_pool,
            tc.tile_pool(name="qk", bufs=HPC) as qk_pool,
            tc.tile_pool(name="v", bufs=HPC) as v_pool,
            tc.tile_pool(name="mpe", bufs=HPC) as mpe_pool,
            tc.tile_pool(name="mdv", bufs=HPC) as mdv_pool,
            tc.tile_pool(name="p", bufs=3) as p_pool,
            tc.tile_pool(name="ot", bufs=5) as ot_pool,
            tc.tile_pool(name="fin", bufs=2) as fin_pool,
            tc.tile_pool(name="spsum", bufs=2, space="PSUM") as s_pool,
            tc.tile_pool(name="opsum", bufs=2, space="PSUM") as o_pool,
            tc.tile_pool(name="fpsum", bufs=2, space="PSUM") as f_pool,
        ):
            ident_sb = const_pool.tile([65, 65], BF16)
            iddr_sb = const_pool.tile([64, 256], DT8)
            warm_sb = const_pool.tile([128, 256], BF16, tag="warm")

            # ACT warmup: memset (gpsimd, write-only) -> exp loads the Exp
            # activation table immediately, overlapping the input DMA wait.
            nc.gpsimd.memset(warm_sb[:], 0.0)
            warm_out = const_pool.tile([1, 1], F32, tag="warmo")
            nc.scalar.activation(
                out=warm_out[:],
                in_=warm_sb[0:1, 0:1],
                func=mybir.ActivationFunctionType.Exp,
            )

            # PE warmup: junk matmuls (dep only on the memset) keep the PE
            # continuously busy from the barrier until real data lands, so
            # the p-state ramp reaches max before the first real QK.
            warm_ps = s_pool.tile([128, S], F32, tag="sps")
            for i in range(WARM_N):
                nc.tensor.matmul(
                    out=warm_ps[:, (i % 8) * 64 : (i % 8) * 64 + 64],
                    lhsT=warm_sb[0:64, 0:128],
                    rhs=warm_sb[0:64, 128 : 128 + 64],
                    start=True,
                    stop=True,
                )

            qk_tiles, v_tiles, mpe_tiles, mdv_tiles = [], [], [], []
            for h in range(HPC):
                qk_tiles.append(qk_pool.tile([64, 2 * S], BF16, name="qk_sb"))
                v_tiles.append(v_pool.tile([128, NT * 80], BF16, name="v_sb"))
                mpe_tiles.append(mpe_pool.tile([64, NPE * 2 * S], DT8, name="mpe_sb"))
                mdv_tiles.append(
                    mdv_pool.tile([128, (NT - NPE) * S], BF16, name="mdv_sb")
                )

            def load_qk(h):
                nc.sync.dma_start(qk_tiles[h][:], qkt[h])

            def load_v(h):
                nc.sync.dma_start(v_tiles[h][:], vaug[h])

            def load_mpe(h):
                nc.sync.dma_start(mpe_tiles[h][:], mpe[h])

            def load_mdv(h, lo, hi):
                nc.sync.dma_start(
                    mdv_tiles[h][:, lo * S : hi * S], mdv[h][:, lo * S : hi * S]
                )

            # prefetch everything; first-needed first.
            load_qk(0)
            load_mpe(0)
            nc.sync.dma_start(ident_sb[:], ident[:])
            nc.sync.dma_start(iddr_sb[:], iddr[:])
            load_v(0)
            load_qk(1)
            load_mdv(0, 0, NT - NPE)
            load_mpe(1)
            load_v(1)
            load_mdv(1, 0, NT - NPE)
            load_qk(2)
            load_mpe(2)
            load_v(2)
            load_mdv(2, 0, NT - NPE)
            load_qk(3)
            load_mpe(3)
            load_v(3)
            load_mdv(3, 0, NT - NPE)

            p_tiles = {}
            o_halves = {}
            av_state = {}

            def start_av(h):
                oa = o_pool.tile([80, 512], F32, tag="ops")
                ob = o_pool.tile([80, 512], F32, tag="ops")
                av_state[h] = (oa, ob)

            def emit_av_chunk(h, half, t):
                o_ps = av_state[h][half]
                nc.tensor.matmul(
                    out=o_ps[:],
                    lhsT=v_tiles[h][:, t * 80 : (t + 1) * 80],
                    rhs=p_tiles[h][
                        :, t * S + half * 512 : t * S + half * 512 + 512
                    ],
                    start=(t == 0),
                    stop=(t == NT - 1),
                )
                if t == NT - 1:
                    # both casts on DVE: ACT stays exp-only
                    ot_sb = ot_pool.tile([80, 512], BF16, name="ot_sb")
                    nc.vector.tensor_copy(ot_sb[:], o_ps[:])
                    o_halves.setdefault(h, []).append(ot_sb)

            def emit_tile(h, t):
                """QK (+ PE mask for PE-class) -> exp -> (DVE mask)."""
                qk_sb = qk_tiles[h]
                s_ps = s_pool.tile([128, S], F32, tag="sps")
                pe_class = t < NPE
                for n in range(2):
                    sl = slice(n * 512, (n + 1) * 512)
                    nc.tensor.matmul(
                        out=s_ps[:, sl],
                        lhsT=qk_sb[:, t * 128 : (t + 1) * 128],
                        rhs=qk_sb[:, NT * 128 + n * 512 : NT * 128 + (n + 1) * 512],
                        start=True,
                        stop=not pe_class,
                    )
                if pe_class:
                    mview = mpe_tiles[h][:].rearrange(
                        "p (t i s) -> p t i s", t=NPE, i=2
                    )
                    for n in range(2):
                        sl = slice(n * 512, (n + 1) * 512)
                        nc.tensor.matmul(
                            out=s_ps[:, sl],
                            lhsT=iddr_sb[:].rearrange("p (i c) -> p i c", i=2),
                            rhs=mview[:, t, :, sl],
                            start=False,
                            stop=True,
                            perf_mode=DR,
                        )
                csl = slice(t * S, (t + 1) * S)
                nc.scalar.activation(
                    out=p_tiles[h][:, csl],
                    in_=s_ps[:],
                    func=mybir.ActivationFunctionType.Exp,
                    scale=0.125,
                )
                if not pe_class:
                    dsl = slice((t - NPE) * S, (t - NPE + 1) * S)
                    nc.vector.tensor_mul(
                        out=p_tiles[h][:, csl],
                        in0=p_tiles[h][:, csl],
                        in1=mdv_tiles[h][:, dsl],
                    )

            def emit_head(h):
                """Tiles of head h with av chunks of head h-1 woven between,
                so the PE always has dependency-free work."""
                p_tiles[h] = p_pool.tile([128, NT * S], BF16, name="p_sb")
                prev = h - 1 if h >= 1 else None
                if h == 0:
                    start_av(0)
                elif h >= 2:
                    start_av(prev)
                for t in range(NT):
                    emit_tile(h, t)
                    if h == 0:
                        # self-weave two tiles behind
                        if t >= 2:
                            emit_av_chunk(0, 0, t - 2)
                            emit_av_chunk(0, 1, t - 2)
                    elif h == 1:
                        # av(0) remainder first (tiles 6,7), then nothing:
                        # av(1) can't start until scores(1) appear
                        if t == 0:
                            for tt in (6, 7):
                                emit_av_chunk(0, 0, tt)
                                emit_av_chunk(0, 1, tt)
                    else:
                        emit_av_chunk(prev, 0, t)
                        emit_av_chunk(prev, 1, t)
                    if h >= 2 and t == 3:
                        emit_tail_transposes(h - 2)
                    if h >= 2 and t == 6:
                        emit_tail_finish(h - 2)

            f_trans = {}

            def emit_tail_transposes(h):
                ot_sbs = o_halves.pop(h)
                f_ps = f_pool.tile([128, 8 * 66], BF16)
                for j in range(8):
                    nc.tensor.transpose(
                        out=f_ps[:, j * 66 : j * 66 + 65],
                        in_=ot_sbs[j // 4][0:65, (j % 4) * 128 : (j % 4 + 1) * 128],
                        identity=ident_sb[:],
                    )
                f_trans[h] = f_ps

            def emit_tail_finish(h):
                p_tiles.pop(h)
                f_ps = f_trans.pop(h)
                fv = f_ps[:].rearrange("p (j c) -> p j c", j=8)
                r_sb = fin_pool.tile([128, 8], F32, tag="rsb8")
                nc.vector.reciprocal(r_sb[:, :, None], fv[:, :, 64:65])
                out_sb = fin_pool.tile([128, 8 * D], BF16, tag="osb8")
                nc.vector.tensor_mul(
                    out=out_sb[:].rearrange("p (j d) -> p j d", j=8),
                    in0=fv[:, :, 0:64],
                    in1=r_sb[:, :, None].to_broadcast((128, 8, D)),
                )
                nc.sync.dma_start(outp[h], out_sb[:])

            for h in range(HPC):
                emit_head(h)
            # last head: self-weave its AV between nothing further; run the
            # remaining chunks directly (exp(t) already emitted), tails of
            # h-2 woven in.
            last = HPC - 1
            start_av(last)
            for t in range(NT):
                emit_av_chunk(last, 0, t)
                emit_av_chunk(last, 1, t)
                if t == 1:
                    emit_tail_transposes(HPC - 2)
                    emit_tail_finish(HPC - 2)
            emit_tail_transposes(last)
            emit_tail_finish(last)

    _hoist_early_dmas(nc)
    _split_multi_waits(nc)
    return nc


def _hoist_early_dmas(nc):
    """Move the first wait-free SP input DMAs from the body basic block to
    the preamble block, ahead of SP's all-engine-barrier wait. Their
    transfers then overlap the ~6us of engine table loads that gate the
    barrier, so the first matmul's operands are resident when the PE wakes.
    Data safety is preserved by the DMAs' own completion semaphores, which
    consumers still wait on."""
    main_bb = body_bb = None
    for name, bb in nc.bb_map.items():
        if name == "main":
            main_bb = bb.bb
        elif len(bb.bb.instructions) > 100:
            body_bb = bb.bb
    if main_bb is None or body_bb is None:
        return
    drain_idx = None
    for idx, inst in enumerate(main_bb.instructions):
        if (
            type(inst).__name__ == "InstDrain"
            and str(getattr(inst, "engine", "")) == "EngineType.SP"
        ):
            drain_idx = idx
            break
    if drain_idx is None:
        return
    hoist = []
    for inst in body_bb.instructions:
        if len(hoist) >= 8:
            break
        if (
            type(inst).__name__ == "InstDMACopy"
            and str(getattr(inst, "engine", "")) == "EngineType.SP"
        ):
            si = getattr(inst, "sync_info", None)
            if si is not None and si.on_wait:
                break  # stop at the first dependent DMA to keep queue order
            hoist.append(inst)
    for inst in hoist:
        body_bb.instructions.remove(inst)
    main_bb.instructions[drain_idx:drain_idx] = hoist


def _split_multi_waits(nc):
    """Walrus's S3_LW codegen can't take >1 sync-wait condition on a Matmult;
    hoist extras into standalone EventSemaphore instructions (same semantics:
    the engine queue stalls on them in program order, like raw-bass wait_ge).

    Before splitting, drop subsumed waits: engine queues execute in program
    order and tile semaphores only count up, so a wait sem>=Y after an
    earlier wait sem>=X (X>=Y) on the same engine is a no-op."""
    for bb in nc.bb_map.values():
        insts = bb.bb.instructions
        seen: dict = {}
        for inst in insts:
            si = getattr(inst, "sync_info", None)
            if si is None or not si.on_wait:
                continue
            eng = getattr(inst, "engine", None)
            e_seen = seen.setdefault(eng, {})
            kept = []
            for cond in si.on_wait:
                if cond.wait_mode == "sem-ge-imm":
                    prev = e_seen.get(cond.id)
                    if prev is not None and prev >= cond.wait_value:
                        continue
                    e_seen[cond.id] = max(prev or 0, cond.wait_value)
                else:
                    # non-monotone wait: stop tracking this semaphore
                    e_seen.pop(cond.id, None)
                kept.append(cond)
            si.on_wait = kept
    for bb in nc.bb_map.values():
        insts = bb.bb.instructions
        new_list = []
        for inst in insts:
            si = getattr(inst, "sync_info", None)
            if (
                si is not None
                and si.on_wait
                and len(si.on_wait) > 1
            ):
                extra = si.on_wait[:-1]
                keep = si.on_wait[-1:]
                for cond in extra:
                    new_list.append(
                        mybir.InstEventSemaphore(
                            name=nc.get_next_instruction_name(),
                            ins=[],
                            outs=[],
                            engine=inst.engine,
                            sync_info=mybir.SyncInfo(on_wait=[cond], on_update=[]),
                        )
                    )
                si.on_wait = keep
            new_list.append(inst)
        insts[:] = new_list


import concourse.bass_utils as _bu

_orig_run_command = _bu.run_command


def _run_command_ldwopt(cmd, **kw):
    if os.environ.get("LDW_OPT") == "1":
        cmd = [
            "--enable-ldw-opt=true" if c == "--enable-ldw-opt=false" else c
            for c in cmd
        ]
    return _orig_run_command(cmd, **kw)


_bu.run_command = _run_command_ldwopt

_NC_CACHE = None


def _get_nc():
    global _NC_CACHE
    if _NC_CACHE is None:
        _NC_CACHE = _build_program()
    return _NC_CACHE


def _make_in_maps(q, k, v, mask):
    q = np.ascontiguousarray(np.asarray(q, dtype=np.float32))
    k = np.ascontiguousarray(np.asarray(k, dtype=np.float32))
    v = np.ascontiguousarray(np.asarray(v, dtype=np.float32))
    mask = np.asarray(mask)
    ident_np = np.eye(65, dtype=ml_dtypes.bfloat16)
    # DR identity: iddr[p, i*128 + c] = 1.0 iff c == i*64 + p
    iddr_np = np.zeros((64, 256), dtype=FP8)
    for i in range(2):
        for p in range(64):
            iddr_np[p, i * 128 + i * 64 + p] = 1.0
    ones_col = np.ones((HPC, S, 1), dtype=np.float32)
    in_maps = []
    for c in range(NCORES):
        sl = slice(c * HPC, (c + 1) * HPC)
        qT = q[sl].transpose(0, 2, 1)  # [HPC, 64, S]
        kT = k[sl].transpose(0, 2, 1)
        qkt_np = np.ascontiguousarray(
            np.concatenate([kT, qT], axis=2)
        ).astype(ml_dtypes.bfloat16)  # [HPC, 64, 2S]: kT tiles then qT
        va = np.concatenate(
            [v[sl], ones_col, np.zeros((HPC, S, 15), np.float32)], axis=2
        )  # [HPC, S, 80]: 64 dims + denominator ones + pad to 80 for XBAR
        vaug_np = np.ascontiguousarray(
            va.reshape(HPC, NT, 128, 80).transpose(0, 2, 1, 3).reshape(HPC, 128, NT * 80)
        ).astype(ml_dtypes.bfloat16)
        mT = mask[sl].transpose(0, 2, 1)  # [HPC, t, s]
        # PE-class tiles (< NPE): additive -240 mask, DR interleaved:
        # mpe[h, p, (t*2 + i)*S + s] = -240 * mT[h, t*128 + i*64 + p, s]
        mpe_np = np.zeros((HPC, 64, NPE * 2 * S), dtype=FP8)
        for t in range(NPE):
            blk = mT[:, t * 128 : (t + 1) * 128, :]  # [HPC, 128, S]
            add = (-240.0 * blk.astype(np.float32)).astype(FP8)
            mpe_np[:, :, (t * 2) * S : (t * 2 + 1) * S] = add[:, 0:64]
            mpe_np[:, :, (t * 2 + 1) * S : (t * 2 + 2) * S] = add[:, 64:128]
        # DVE-class tiles: multiplicative keep in bf16
        mdv_np = np.ascontiguousarray(
            (~mT[:, NPE * 128 :, :])
            .reshape(HPC, NT - NPE, 128, S)
            .transpose(0, 2, 1, 3)
            .reshape(HPC, 128, (NT - NPE) * S)
        ).astype(ml_dtypes.bfloat16)
        in_maps.append(
            {
                "qkt": qkt_np,
                "vaug": vaug_np,
                "mpe": mpe_np,
                "mdv": mdv_np,
                "ident": ident_np,
                "iddr": iddr_np,
            }
        )
    return in_maps


def _gather(results):
    outs = []
    for c in range(NCORES):
        o = np.asarray(results[c]["outp"], dtype=np.float32)  # [HPC,128,NT*D]
        o = o.reshape(HPC, 128, NT, D).transpose(0, 2, 1, 3).reshape(HPC, S, D)
        outs.append(o)
    return np.ascontiguousarray(np.concatenate(outs, axis=0))


def _install_profile_shim():
    """The agent image's antenv lacks axon_hooks; recreate it from the boot
    module's ctypes implementation so trace=True can capture NTFF profiles."""
    import types

    if "antenv.axon_hooks" in sys.modules:
        return
    try:
        from trn_agent_boot.trn_boot import _ntff_profile_via_ctypes

        hook = _ntff_profile_via_ctypes("/opt/axon/libaxon_pjrt.so")
        mod = types.ModuleType("antenv.axon_hooks")
        mod.get_axon_ntff_profile_hook = lambda: hook
        mod.set_axon_ntff_profile_hook = lambda h: None
        sys.modules["antenv.axon_hooks"] = mod
        # don't try to copy artifacts to a remote bucket from the sandbox
        import concourse.bass_utils as _bu

        _bu.upload_artifacts = lambda tmpdir: tmpdir
    except Exception as e:  # profiling is best-effort
        print(f"profile shim unavailable: {e}", file=sys.stderr)


def run(q, k, v, mask, trace=False, **kw):
    nc = _get_nc()
    if trace:
        _install_profile_shim()
    in_maps = _make_in_maps(q, k, v, mask)
    res = run_bass_kernel_spmd(nc, in_maps, list(range(NCORES)), trace=trace, **kw)
    return _gather(res.results), res


def kernel(q, k, v, mask):
    out, _ = run(q, k, v, mask)
    return out
